# revision 17
# baseline (speedup 1.0000x reference)
"""Trainium2 Bass kernel for nn_Attention (general attention + output projection).

Computation (per batch b):
    tgt_p = tgt @ W_general.T
    score = tgt_p @ src.T                      (t x s)
    score = where(arange(s) < len[b], score, -inf)
    P     = softmax(score, axis=-1)            -> output "align"
    c     = P @ src
    attn  = concat([c, tgt], -1) @ W_out.T     -> output "attn"

Sharding: batch 8 == cores 8, one batch per core, pure SPMD (no collectives).

On-chip algorithm (per core), re-associated to keep natural layouts:
    src_p^T[d,s] = sum_e Wg[e,d] * srcT[e,s]      (resident SBUF, 8MB, f32r)
    src_o[s,j]   = sum_d srcT[d,s] * Wo1T[d,j]    (resident SBUF, 4MB, fp16)
    per 128-row t-tile (software-pipelined, score(n+1) hides softmax(n)):
      score[t,s] = sum_d tgtT[d,t] * src_p^T[d,s]   (PSUM, f32r matmuls)
      masked softmax in [t,s] layout: DVE mask-add + per-chunk partial max,
      ACT exp(bias=-max, accum_out=rowsum), normalize; fp16 copy of P
      P^T tiles via fp16 PE transposes (1 cyc/row)
      attn[t,j]  = sum_s P^T[s,t]*src_o[s,j] (fp16)
                 + sum_d tgtT[d,t]*Wo2T[d,j] (f32r), one PSUM accumulation

All matmuls run at the PE's full rate (f32r/tf32 or fp16, 1 cycle/row);
exact-fp32 would be 4x slower. Precision on HW: rel err ~1.2e-3.
"""

import numpy as np
import ml_dtypes

BZ, S, T, D = 8, 2048, 2048, 1024
P128 = 128
DC = D // 128    # 8 contraction chunks over features
SC = S // 128    # 16 source chunks
TC = T // 128    # 16 target tiles
NJ = D // 512    # 2 output column chunks
SB = S // 512    # 4 score column chunks
NEG = -1.0e30

# score-chain matmul dtype: "f32r" (fast, reduced precision) or "f32" (exact, 4x slower)
SCORE_DTYPE = "f32r"

_cache = {}


def _build(reps=1):
    from concourse import bacc, mybir
    import concourse.tile as tile
    from concourse.masks import make_identity

    f32 = mybir.dt.float32
    f32r = mybir.dt.float32r
    f16 = mybir.dt.float16
    sdt = f32r if SCORE_DTYPE == "f32r" else f32

    nc = bacc.Bacc("TRN2", target_bir_lowering=False, debug=False, num_devices=8)
    srcT = nc.dram_tensor("srcT", [D, S], f32, kind="ExternalInput").ap()
    tgtT = nc.dram_tensor("tgtT", [D, T], f32, kind="ExternalInput").ap()
    wg = nc.dram_tensor("wg", [D, D], f32, kind="ExternalInput").ap()
    woT = nc.dram_tensor("woT", [2 * D, D], f32, kind="ExternalInput").ap()
    maskrep = nc.dram_tensor("maskrep", [P128, S], mybir.dt.bfloat16, kind="ExternalInput").ap()
    align = nc.dram_tensor("align", [T, S], f32, kind="ExternalOutput").ap()
    attn = nc.dram_tensor("attn", [T, D], f32, kind="ExternalOutput").ap()

    srcT_v = srcT.rearrange("(c p) s -> p c s", p=P128)   # [128, 8, 2048]
    tgtT_v = tgtT.rearrange("(c p) t -> p c t", p=P128)   # [128, 8, 2048]
    wg_v = wg.rearrange("(c p) d -> p c d", p=P128)       # [128, 8, 1024], rows = e
    wo1_v = woT[0:D].rearrange("(c p) j -> p c j", p=P128)
    wo2_v = woT[D:2 * D].rearrange("(c p) j -> p c j", p=P128)

    Exp = mybir.ActivationFunctionType.Exp
    Copy = mybir.ActivationFunctionType.Copy
    AxX = mybir.AxisListType.X
    Max = mybir.AluOpType.max

    with tile.TileContext(nc) as tc:
      for _rep in range(reps):
        with (
            tc.tile_pool(name="res", bufs=1) as res,
            tc.tile_pool(name="psum", bufs=1, space="PSUM") as psum,
            tc.tile_pool(name="small", bufs=2) as small,
            tc.tile_pool(name="ptg", bufs=4) as ptg,
        ):
            srcp = [res.tile([P128, S], f32, tag=f"srcp{i}", name=f"srcp{i}")
                    for i in range(DC)]
            srco = [res.tile([P128, D], f16, tag=f"srco{i}", name=f"srco{i}")
                    for i in range(SC)]
            mask_sb = res.tile([P128, S], mybir.dt.bfloat16, tag="mask", name="mask_sb")
            ident = res.tile([P128, P128], f16, tag="ident", name="ident")
            make_identity(nc, ident)

            tgs = {}

            def load_tg2(tn, _ptg=None):
                pass  # replaced below

            def _mk_load_tg(ptg_pool, tgtT_view, dt):
                def f(tn):
                    if tn >= TC or tn in tgs:
                        return
                    tg = ptg_pool.tile([P128, DC, P128], f32, tag="tg", name="tg")
                    nc.sync.dma_start(out=tg.bitcast(dt), in_=tgtT_view[:, :, tn * 128:(tn + 1) * 128].bitcast(dt))
                    tgs[tn] = tg
                return f

            load_tg2 = _mk_load_tg(ptg, tgtT_v, sdt)

            # ---- Phase A: precompute src_p^T (resident) and src_o (resident) ----
            with (
                tc.tile_pool(name="pa", bufs=2) as pa,
                tc.tile_pool(name="paw", bufs=4) as paw,
            ):
                # wg in two d-column halves, chunk-loaded per e-chunk so the
                # first A1 matmul starts as soon as chunk (h=0, ec=0) lands.
                wg_h = []
                blk0 = None
                for h in range(2):
                    wgt = paw.tile([P128, DC, 512], f32, tag="w", name=f"wg_h{h}")
                    for ec in range(DC):
                        nc.sync.dma_start(
                            out=wgt[:, ec, :].bitcast(sdt),
                            in_=wg_v[:, ec, h * 512:(h + 1) * 512].bitcast(sdt))
                    wg_h.append(wgt)
                    if h == 0:
                        blk0 = pa.tile([P128, DC, 256], f32, tag="blk", name="blk")
                        nc.sync.dma_start(
                            out=blk0.bitcast(sdt),
                            in_=srcT_v[:, :, 0:256].bitcast(sdt))
                # wo1 halves prefetch into spare "w" slots mid-A1
                sls = {}

                def load_sl(sn):
                    sl = pa.tile([P128, DC, P128], f32, tag="sl", name="sl")
                    nc.sync.dma_start(
                        out=sl.bitcast(f32r),
                        in_=srcT_v[:, :, sn * 128:(sn + 1) * 128].bitcast(f32r))
                    sls[sn] = sl
                    return sl

                wo1_h = [None, None]

                def load_wo1(h):
                    wot = paw.tile([P128, DC, 512], f32, tag="w", name=f"wo1_h{h}")
                    nc.sync.dma_start(
                        out=wot.bitcast(f32r),
                        in_=wo1_v[:, :, h * 512:(h + 1) * 512].bitcast(f32r))
                    wo1_h[h] = wot

                # A1: src_p^T[d, s] resident (stream srcT in 256-col blocks)
                for sb in range(S // 256):
                    if sb == 0:
                        blk = blk0
                    else:
                        blk = pa.tile([P128, DC, 256], f32, tag="blk", name="blk")
                        nc.sync.dma_start(
                            out=blk.bitcast(sdt),
                            in_=srcT_v[:, :, sb * 256:(sb + 1) * 256].bitcast(sdt))
                    if sb == 2:
                        nc.sync.dma_start(out=mask_sb, in_=maskrep)
                    if sb == 3:
                        load_wo1(0)
                    if sb == 5:
                        load_wo1(1)
                    if sb in (6, 7):
                        load_sl(sb - 6)
                    for dc in range(DC):
                        h, hc = dc // 4, dc % 4
                        ps = psum.tile([P128, 256], f32, tag="mm", bufs=4, name="ps_a1")
                        for ec in range(DC):
                            nc.tensor.matmul(
                                ps,
                                lhsT=wg_h[h][:, ec, hc * 128:(hc + 1) * 128].bitcast(sdt),
                                rhs=blk[:, ec, :].bitcast(sdt),
                                start=(ec == 0), stop=(ec == DC - 1),
                            )
                        nc.vector.tensor_copy(
                            out=srcp[dc][:, sb * 256:(sb + 1) * 256].bitcast(sdt), in_=ps)
                # A2: src_o[s, j] resident; wo1 halves go into the two freed
                # "w" slots, nj-inner so each sl slice is read once.
                for sn in range(SC):
                    if sn in sls:
                        sl = sls[sn]
                    else:
                        sl = load_sl(sn)
                    if sn == SC - 2:
                        load_tg2(0)
                    if sn == SC - 1:
                        load_tg2(1)
                    for nj in range(NJ):
                        ps = psum.tile([P128, 512], f32, tag="mm", bufs=4, name="ps_a2")
                        for dc in range(DC):
                            nc.tensor.matmul(
                                ps,
                                lhsT=sl[:, dc, :].bitcast(f32r),
                                rhs=wo1_h[nj][:, dc, :].bitcast(f32r),
                                start=(dc == 0), stop=(dc == DC - 1),
                            )
                        nc.vector.tensor_copy(
                            out=srco[sn][:, nj * 512:(nj + 1) * 512], in_=ps)

            # ---- Phase B: per-t-tile score -> softmax -> transpose -> attn ----
            # wo2 loads into space freed by the phase-A pools; the tgt-half of
            # the output projection accumulates into the attn PSUM directly.
            with (
                tc.tile_pool(name="pw2", bufs=1) as pw2,
                tc.tile_pool(name="pb", bufs=2) as pb,
            ):
                wo2_sb = pw2.tile([P128, DC, D], f32, tag="w2", name="wo2_sb")

                def load_wo2(h):
                    nc.sync.dma_start(
                        out=wo2_sb[:, :, h * 512:(h + 1) * 512].bitcast(sdt),
                        in_=wo2_v[:, :, h * 512:(h + 1) * 512].bitcast(sdt))
                state = {}

                def stage_score(tn):
                    tg = tgs.pop(tn)
                    load_tg2(tn + 2)
                    scm = pb.tile([P128, S], f32, tag="scm", name="scm")
                    mxp = small.tile([P128, SB], f32, tag="mxp", name="mxp")
                    for sb in range(SB):
                        ps = psum.tile([P128, 512], f32, tag="mm", bufs=4, name="ps_sc")
                        for dc in range(DC):
                            nc.tensor.matmul(
                                ps,
                                lhsT=tg[:, dc, :].bitcast(sdt),
                                rhs=srcp[dc][:, sb * 512:(sb + 1) * 512].bitcast(sdt),
                                start=(dc == 0), stop=(dc == DC - 1),
                            )
                        nc.vector.tensor_add(
                            out=scm[:, sb * 512:(sb + 1) * 512], in0=ps,
                            in1=mask_sb[:, sb * 512:(sb + 1) * 512])
                        nc.vector.tensor_reduce(
                            mxp[:, sb:sb + 1], scm[:, sb * 512:(sb + 1) * 512],
                            axis=AxX, op=Max)
                    negmx = small.tile([P128, 1], f32, tag="negmx", name="negmx")
                    nc.vector.tensor_reduce(negmx, mxp, axis=AxX, op=Max, negate=True)
                    rowsum = small.tile([P128, 1], f32, tag="rowsum", name="rowsum")
                    nc.scalar.activation(out=scm, in_=scm, func=Exp,
                                         bias=negmx, scale=1.0, accum_out=rowsum)
                    rs = small.tile([P128, 1], f32, tag="rs", name="rs")
                    nc.vector.reciprocal(rs, rowsum)
                    scm16 = pb.tile([P128, S], f16, tag="scm16", name="scm16")
                    nc.scalar.activation(out=scm16, in_=scm, func=Copy, scale=rs)
                    nc.scalar.activation(out=scm, in_=scm, func=Copy, scale=rs)
                    nc.sync.dma_start(out=align[tn * 128:(tn + 1) * 128, :], in_=scm)
                    state[tn] = (scm16, tg)

                def stage_attn(tn):
                    scm16, tg = state.pop(tn)
                    pt = pb.tile([P128, SC, P128], f16, tag="pt", name="pt", bufs=2)
                    for g in range(4):
                        pst = psum.tile([P128, 512], f16, tag="tr", bufs=2, name="ps_tr")
                        for q in range(4):
                            si = g * 4 + q
                            nc.tensor.transpose(
                                out=pst[:, q * 128:(q + 1) * 128],
                                in_=scm16[:, si * 128:(si + 1) * 128],
                                identity=ident)
                        nc.vector.tensor_copy(
                            out=pt[:, g * 4:(g + 1) * 4, :].rearrange("p a b -> p (a b)"),
                            in_=pst)
                    outt = pb.tile([P128, D], f32, tag="outt", name="outt")
                    for nj in range(NJ):
                        psa = psum.tile([P128, 512], f32, tag="at", bufs=2, name="ps_at")
                        for si in range(SC):
                            nc.tensor.matmul(
                                psa,
                                lhsT=pt[:, si, :],
                                rhs=srco[si][:, nj * 512:(nj + 1) * 512],
                                start=(si == 0), stop=False,
                            )
                        for dc in range(DC):
                            nc.tensor.matmul(
                                psa,
                                lhsT=tg[:, dc, :].bitcast(sdt),
                                rhs=wo2_sb[:, dc, nj * 512:(nj + 1) * 512].bitcast(sdt),
                                start=False, stop=(dc == DC - 1),
                            )
                        nc.vector.tensor_copy(out=outt[:, nj * 512:(nj + 1) * 512],
                                              in_=psa)
                    nc.sync.dma_start(out=attn[tn * 128:(tn + 1) * 128, :], in_=outt)

                for tn in range(TC):
                    stage_score(tn)
                    if tn < 2:
                        load_wo2(tn)
                    if tn > 0:
                        stage_attn(tn - 1)
                stage_attn(TC - 1)

    nc.compile()
    return nc


def _get_program(reps=1):
    key = f"nc{reps}"
    if key not in _cache:
        _cache[key] = _build(reps)
    return _cache[key]


def _make_in_maps(src, tgt, W_general, W_out, src_lengths, src_max_len):
    src = np.ascontiguousarray(np.asarray(src, dtype=np.float32))
    tgt = np.ascontiguousarray(np.asarray(tgt, dtype=np.float32))
    W_general = np.ascontiguousarray(np.asarray(W_general, dtype=np.float32))
    W_out = np.ascontiguousarray(np.asarray(W_out, dtype=np.float32))
    lengths = np.asarray(src_lengths).astype(np.int64).reshape(BZ)
    smax = int(np.asarray(src_max_len))

    woT = np.ascontiguousarray(W_out.T)
    arange = np.arange(S)
    in_maps = []
    for b in range(BZ):
        maskrow = np.where(arange < lengths[b], 0.0, NEG).astype(ml_dtypes.bfloat16)
        if smax < S:
            maskrow[smax:] = NEG
        in_maps.append({
            "srcT": np.ascontiguousarray(src[b].T),
            "tgtT": np.ascontiguousarray(tgt[b].T),
            "wg": W_general,
            "woT": woT,
            "maskrep": np.ascontiguousarray(np.broadcast_to(maskrow, (P128, S))),
        })
    return in_maps


def kernel(src, tgt, W_general, W_out, src_lengths, src_max_len):
    from concourse.bass_utils import run_bass_kernel_spmd

    in_maps = _make_in_maps(src, tgt, W_general, W_out, src_lengths, src_max_len)
    nc = _get_program()
    res = run_bass_kernel_spmd(nc, in_maps, list(range(BZ)))
    attn_h = np.stack([res.results[b]["attn"] for b in range(BZ)])
    align_score = np.stack([res.results[b]["align"] for b in range(BZ)])
    return attn_h, align_score
